# revision 31
# baseline (speedup 1.0000x reference)
"""AdditiveAttention Trainium2 kernel (8 NeuronCores, data-parallel over batch).

Math: scores[b,q,k] = sum_h wv[h] * tanh(qp[b,q,h] + kp[b,k,h]) with
qp = queries @ Wq^T, kp = keys @ Wk^T, then length-masked softmax over k and
attn @ values.

Device strategy (per core, 2 batch slots):
  tanh(x) ~= sum_t c_t sin(w_t x), w_t = (2t-1)*w0 (odd harmonics, fit under
  the N(0,2) distribution of qp+kp). sin(w(a+b)) = sin(wa)cos(wb) +
  cos(wa)sin(wb) turns scoring into matmuls with contraction 2*T*H.
  All harmonics come from one in-domain ACT Sin pair (cos via a pi/2 bias)
  through the odd-step Chebyshev recurrence f_{k+2} = 2cos(2w0 x) f_k -
  f_{k-2}, computed on [qp;kp]-stacked tiles with the sin|cos halves merged
  so each ladder step is two DVE ops feeding the score matmuls per harmonic.
  Softmax needs no max pass (scores are bounded); the 0/1 length mask and the
  ones-column that produces Z are folded into V on the host, so softmax is
  exp -> AV-matmul -> scale by 1/Z. Inputs arrive as one packed per-partition
  blob -> few large contiguous DMAs.
"""

import os
import sys

for _p in ("/opt/trn_rl_repo", os.path.expanduser("~/.axon_site/_ro/trn_rl_repo")):
    if os.path.isdir(_p) and _p not in sys.path:
        sys.path.insert(0, _p)

import math

import ml_dtypes
import numpy as np

import concourse.bass as bass
import concourse.mybir as mybir
import concourse.tile as tile
from concourse import bacc
from concourse.bass_utils import run_bass_kernel_spmd

BF16 = ml_dtypes.bfloat16
F32 = mybir.dt.float32
BF = mybir.dt.bfloat16

B, Q, K, H = 16, 512, 512, 64
DQ = DK = DV = 256
P = 128
NCORES = 8
SLOTS = 2

W0 = 0.4310
CS = np.array([1.181119, 0.230435, 0.05738, 0.017487], np.float64)
T = len(CS)

AF = mybir.ActivationFunctionType
ALU = mybir.AluOpType

_COMPILE_CACHE = {}

TRACE = False
LAST_RESULTS = None


def _offsets(kt_bounds):
    """Per-partition element offsets inside the packed bf16 blob."""
    off = {}
    o = 0
    off["wq"] = o
    o += 2 * H
    off["wk"] = o
    o += 2 * H
    for s in range(SLOTS):
        off[f"q{s}"] = o
        o += 2 * Q
        off[f"k{s}"] = o
        o += 2 * K
    for s in range(SLOTS):
        off[f"v{s}"] = o
        o += (DV + 1) * kt_bounds[s]
    off["end"] = o
    return off


def _build(kt_bounds):
    nc = bacc.Bacc()
    off = _offsets(kt_bounds)
    XB = off["end"]

    ib = nc.declare_dram_parameter("ib", [P, XB], BF, isOutput=False)
    cwv = nc.declare_dram_parameter("cwv", [P, T], F32, isOutput=False)
    out = nc.declare_dram_parameter("out", [SLOTS, Q, DV], F32, isOutput=True)

    with tile.TileContext(nc) as tc:
        with (
            tc.tile_pool(name="singles", bufs=1) as singles,
            tc.tile_pool(name="lad", bufs=2) as lad,
            tc.tile_pool(name="feat", bufs=2) as feat,
            tc.tile_pool(name="esb", bufs=2) as esb,
            tc.tile_pool(name="osb", bufs=4) as osb,
            tc.tile_pool(name="pproj", bufs=2, space="PSUM") as pproj,
            tc.tile_pool(name="psc", bufs=4, space="PSUM") as psc,
            tc.tile_pool(name="pav", bufs=2, space="PSUM") as pav,
        ):
            dw = singles.tile([P, Q], BF)
            nc.vector.memset(dw[:], 0.0)
            warm_ps = pav.tile([P, Q], F32, tag="o_ps")
            for _ in range(10):
                nc.tensor.matmul(warm_ps[:], dw[:, 0:P], dw[:], start=True, stop=True)

            ib_sb = singles.tile([P, XB], BF)
            cuts = [0, off["q0"], off["k0"], off["q1"], off["k1"],
                    off["v0"], off["v0"] + (off["v1"] - off["v0"]) // 2,
                    off["v1"], XB]
            for j in range(len(cuts) - 1):
                eng = nc.sync if j % 2 == 0 else nc.scalar
                eng.dma_start(ib_sb[:, cuts[j]:cuts[j + 1]], ib[:, cuts[j]:cuts[j + 1]])
            cwv_sb = singles.tile([P, T], F32)
            nc.sync.dma_start(cwv_sb[:], cwv[:, :])
            pi2 = singles.tile([P, 1], F32)
            nc.vector.memset(pi2[:], math.pi / 2)

            wq_v = ib_sb[:, off["wq"] : off["wq"] + 2 * H].rearrange(
                "p (c h) -> p c h", c=2
            )
            wk_v = ib_sb[:, off["wk"] : off["wk"] + 2 * H].rearrange(
                "p (c h) -> p c h", c=2
            )

            va_v, sc_ps = [None] * SLOTS, [None] * SLOTS
            deferred_cwv = []

            qks = [None] * SLOTS
            for s in range(SLOTS):
                q_v = ib_sb[:, off[f"q{s}"] : off[f"q{s}"] + 2 * Q].rearrange(
                    "p (c q) -> p c q", c=2
                )
                k_v = ib_sb[:, off[f"k{s}"] : off[f"k{s}"] + 2 * K].rearrange(
                    "p (c k) -> p c k", c=2
                )
                va_v[s] = ib_sb[
                    :, off[f"v{s}"] : off[f"v{s}"] + (DV + 1) * kt_bounds[s]
                ].rearrange("p (kt v) -> p kt v", kt=kt_bounds[s])

                qk = pproj.tile([P, Q], F32, tag="qk")
                for c in range(2):
                    nc.tensor.matmul(
                        qk[0:H, :], wq_v[:, c, :], q_v[:, c, :],
                        start=(c == 0), stop=(c == 1), tile_position=(0, 0),
                    )
                for c in range(2):
                    nc.tensor.matmul(
                        qk[H:P, :], wk_v[:, c, :], k_v[:, c, :],
                        start=(c == 0), stop=(c == 1), tile_position=(0, H),
                    )
                qks[s] = qk

            for s in range(SLOTS):
                ktn = kt_bounds[s]
                qk = qks[s]

                LD = lad.tile([P, T, 2 * Q], BF, tag="LD")
                sq1 = lad.tile([P, Q], BF, tag="sq1", name=f"sq1_{s}")
                c2d = lad.tile([P, 2 * Q], BF, tag="c2d")
                m1 = lad.tile([P, 2 * Q], BF, tag="m1")

                nc.scalar.activation(LD[:, 0, 0:Q], qk[:], AF.Sin, scale=W0)
                nc.scalar.activation(
                    LD[:, 0, Q : 2 * Q], qk[:], AF.Sin, scale=W0, bias=pi2[:]
                )
                nc.vector.tensor_tensor(
                    sq1[:], LD[:, 0, 0:Q], LD[:, 0, 0:Q], ALU.mult
                )
                # 2cos(2w0 x) duplicated over the sin|cos halves
                nc.vector.tensor_scalar(c2d[:, 0:Q], sq1[:], -4.0, 2.0, ALU.mult, ALU.add)
                nc.vector.tensor_scalar(
                    c2d[:, Q : 2 * Q], sq1[:], -4.0, 2.0, ALU.mult, ALU.add
                )
                nc.vector.tensor_scalar(m1[:, 0:Q], sq1[:], -4.0, 3.0, ALU.mult, ALU.add)
                nc.vector.tensor_scalar(
                    m1[:, Q : 2 * Q], sq1[:], -4.0, 1.0, ALU.mult, ALU.add
                )

                fa = feat.tile([P, T, Q], BF, tag="fa")
                fb = feat.tile([P, T, Q], BF, tag="fb")

                def asm(t, fa=fa, fb=fb, LD=LD):
                    nc.sync.dma_start(fa[0:H, t, :], LD[0:H, t, 0:Q])
                    nc.gpsimd.dma_start(fa[H:P, t, :], LD[0:H, t, Q : 2 * Q])
                    nc.gpsimd.dma_start(fb[0:H, t, :], LD[H:P, t, Q : 2 * Q])
                    nc.sync.dma_start(fb[H:P, t, :], LD[H:P, t, 0:Q])

                def cwv_scale(t, fb=fb):
                    deferred_cwv.append((fb, t))

                nc.vector.tensor_tensor(LD[:, 1, :], m1[:], LD[:, 0, :], ALU.mult)
                asm(0)
                for t in range(2, T):
                    tmp = lad.tile([P, 2 * Q], BF, tag="ltmp")
                    nc.vector.tensor_tensor(tmp[:], c2d[:], LD[:, t - 1, :], ALU.mult)
                    nc.vector.tensor_tensor(
                        LD[:, t, :], tmp[:], LD[:, t - 2, :], ALU.subtract
                    )
                    asm(t - 1)
                    cwv_scale(t - 2)
                asm(T - 1)
                cwv_scale(T - 2)
                cwv_scale(T - 1)

                sc_ps[s] = [psc.tile([P, Q], F32, tag="sc", name=f"sc{s}_{kt}")
                            for kt in range(ktn)]
                for base in range(0, ktn, 2):
                    kts = range(base, min(base + 2, ktn))
                    for t in range(T):
                        for kt in kts:
                            nc.tensor.matmul(
                                sc_ps[s][kt][:],
                                fb[:, t, kt * P : (kt + 1) * P],
                                fa[:, t, :],
                                start=(t == 0),
                                stop=(t == T - 1),
                            )

            # cwv scaling on ACT (Copy with per-partition scale), emitted
            # after all sins so the scalar stream never head-blocks on the
            # ladder-gated assembly DMAs
            for fb_, t_ in deferred_cwv:
                nc.scalar.activation(
                    fb_[:, t_, :], fb_[:, t_, :], AF.Copy,
                    scale=cwv_sb[:, t_ : t_ + 1],
                )

            # prefetch the exp table after the trig-table work is queued
            # (input = last sq1 so the scheduler cannot hoist it before them)
            dxe = singles.tile([P, 1], BF)
            nc.scalar.activation(dxe[:], sq1[:, 0:1], AF.Exp)

            for s in range(SLOTS):
                ktn = kt_bounds[s]
                e_tiles = []
                for kt in range(ktn):
                    e_kt = esb.tile([P, Q], BF, tag=f"e{kt}")
                    nc.scalar.activation(e_kt[:], sc_ps[s][kt][:], AF.Exp)
                    e_tiles.append(e_kt)
                for qt in range(Q // P):
                    o_ps = pav.tile([P, DV + 1], F32, tag="o_ps")
                    for kt in range(ktn):
                        nc.tensor.matmul(
                            o_ps[:],
                            e_tiles[kt][:, qt * P : (qt + 1) * P],
                            va_v[s][:, kt, :],
                            start=(kt == 0),
                            stop=(kt == ktn - 1),
                        )
                    rz = osb.tile([P, 1], F32, tag="rz")
                    nc.vector.reciprocal(rz[:], o_ps[:, DV : DV + 1])
                    o_sb = osb.tile([P, DV], F32, tag="o_sb")
                    nc.vector.tensor_scalar_mul(o_sb[:], o_ps[:, 0:DV], rz[:])
                    eng = nc.sync if qt % 2 == 0 else nc.scalar
                    eng.dma_start(out[s, qt * P : (qt + 1) * P, :], o_sb[:])

    nc.finalize()
    return nc


def kernel(queries, keys, values, valid_lens, Wq, Wk, wv):
    global LAST_RESULTS
    queries = np.asarray(queries, np.float32)
    keys = np.asarray(keys, np.float32)
    values = np.asarray(values, np.float32)
    vl = np.asarray(valid_lens).astype(np.int64)
    Wq = np.asarray(Wq, np.float32)
    Wk = np.asarray(Wk, np.float32)
    wv = np.asarray(wv, np.float32)

    order = np.argsort(-vl, kind="stable")
    slot_b = [order[:NCORES], order[NCORES:]]
    kt_bounds = tuple(max(1, math.ceil(int(vl[sb].max()) / P)) for sb in slot_b)

    if kt_bounds not in _COMPILE_CACHE:
        _COMPILE_CACHE[kt_bounds] = _build(kt_bounds)
    nc = _COMPILE_CACHE[kt_bounds]
    off = _offsets(kt_bounds)
    XB = off["end"]

    # host-side packing --------------------------------------------------
    def chunked(mat, d_in, width):
        # [d_in, width] -> [128, nchunks*width] with chunk-major per partition
        n = d_in // P
        return (
            mat.reshape(n, P, width).transpose(1, 0, 2).reshape(P, n * width)
        )

    mask = (np.arange(K)[None, :] < vl[:, None]).astype(np.float32)  # [B, K]
    vaug = np.concatenate(
        [values * mask[:, :, None], mask[:, :, None]], axis=2
    )  # [B, K, 257]

    qT = np.ascontiguousarray(queries.transpose(0, 2, 1))  # [B, 256, 512]
    kT = np.ascontiguousarray(keys.transpose(0, 2, 1))

    wq_p = chunked(np.ascontiguousarray(Wq.T), DQ, H)  # [128, 128]
    wk_p = chunked(np.ascontiguousarray(Wk.T), DK, H)

    blobs = np.empty((NCORES, P, XB), BF16)
    for i in range(NCORES):
        for s in range(SLOTS):
            b = int(slot_b[s][i])
            ktn = kt_bounds[s]
            blobs[i, :, off[f"q{s}"] : off[f"q{s}"] + 2 * Q] = chunked(
                qT[b], DQ, Q
            )
            blobs[i, :, off[f"k{s}"] : off[f"k{s}"] + 2 * K] = chunked(
                kT[b], DK, K
            )
            blobs[i, :, off[f"v{s}"] : off[f"v{s}"] + (DV + 1) * ktn] = (
                vaug[b, : ktn * P]
                .reshape(ktn, P, DV + 1)
                .transpose(1, 0, 2)
                .reshape(P, ktn * (DV + 1))
            )
        blobs[i, :, off["wq"] : off["wq"] + 2 * H] = wq_p
        blobs[i, :, off["wk"] : off["wk"] + 2 * H] = wk_p

    cwv_h = (CS[None, :] * wv[:, None].astype(np.float64)).astype(np.float32)
    cwv_full = np.concatenate([cwv_h, cwv_h], axis=0)  # [128, T]

    in_maps = [{"ib": blobs[i], "cwv": cwv_full} for i in range(NCORES)]

    res = None
    last_exc = None
    for attempt in range(3):
        try:
            res = run_bass_kernel_spmd(
                nc, in_maps, core_ids=list(range(NCORES)), trace=TRACE
            )
            _ = np.asarray(res.results[0]["out"])  # force device->host now
            break
        except Exception as exc:  # transient device wedge: retry
            last_exc = exc
            res = None
    if res is None:
        raise last_exc
    LAST_RESULTS = res

    out = np.empty((B, Q, DV), np.float32)
    for i in range(NCORES):
        o = np.asarray(res.results[i]["out"])
        out[slot_b[0][i]] = o[0]
        out[slot_b[1][i]] = o[1]
    return out
